# revision 1
# baseline (speedup 1.0000x reference)
"""BiLSTM Trainium2 kernel: B=64, T=512, D=256, H=256, 8 NeuronCores.

Sharding: batch 8-way (8 sequences per core). Each core runs BOTH
directions (forward + backward) as two independent recurrent chains so
the engines can interleave them (one chain's elementwise hides under the
other chain's matmuls).

Host (numpy) does all data movement that is pure layout: per-length
sequence reversal for the backward direction, transposes into the
[feature-on-partition] layouts the device wants, gate reordering
(i,f,g,o) -> (i,f,o,g) so sigmoid/tanh each cover one contiguous column
range, bias folding, and the final gather/mask/concat.

Device (per core):
  phase 1: x projection  xpT[g,(t,b)] = WihT.T @ xT  (+bias, bf16, SBUF-resident)
  phase 2: 512-step recurrence, per step per direction:
      gates.T[128x64] += Whh tiles (16 matmuls, weight-stationary bf16)
      sigmoid/tanh + c/h update on [128,16..64] tiles (DVE+ACT)
  h states staged 32 steps at a time, DMA'd to DRAM as bf16.
"""

import sys

for _p in ("/opt/trn_rl_repo",):
    if _p not in sys.path:
        sys.path.insert(0, _p)

import numpy as np
import ml_dtypes

import concourse.bass as bass
import concourse.mybir as mybir
import concourse.tile as tile
from concourse.tile import add_dep_helper
from concourse import bacc
from concourse.bass_utils import run_bass_kernel_spmd

B, T, D, H = 64, 512, 256, 256
NCORES = 8
BC = B // NCORES          # 8 sequences per core
G4 = 4 * H                # 1024 gate dims
STG = 32                  # recurrence steps per output staging block

BF16 = mybir.dt.bfloat16
F32 = mybir.dt.float32
AF = mybir.ActivationFunctionType

# gate reorder (torch i,f,g,o) -> (i,f,o,g)
_PERM = np.concatenate(
    [np.arange(0, H), np.arange(H, 2 * H), np.arange(3 * H, 4 * H), np.arange(2 * H, 3 * H)]
)


def build_nc(t_steps=T):
    assert t_steps % STG == 0
    nb = t_steps // STG
    TB = t_steps * BC  # (t,b) columns per k-half of xT

    nc = bacc.Bacc(None, target_bir_lowering=False)

    xt_d, wih_d, whh_d, bias_d = {}, {}, {}, {}
    for d in ("f", "b"):
        xt_d[d] = nc.dram_tensor(f"xt_{d}", [128, 2 * TB], BF16, kind="ExternalInput")
        wih_d[d] = nc.dram_tensor(f"wih_{d}", [128, 2048], BF16, kind="ExternalInput")
        whh_d[d] = nc.dram_tensor(f"whh_{d}", [128, 2048], BF16, kind="ExternalInput")
        bias_d[d] = nc.dram_tensor(f"bias_{d}", [128, 8], F32, kind="ExternalInput")
    ident_d = nc.dram_tensor("ident", [128, 128], BF16, kind="ExternalInput")
    out_e = nc.dram_tensor("out", [128, t_steps * 4 * BC], BF16, kind="ExternalOutput")

    with tile.TileContext(nc) as tc:
        with (
            tc.tile_pool(name="big", bufs=1) as big,
            tc.tile_pool(name="work", bufs=3) as work,
            tc.tile_pool(name="stgp", bufs=2) as stgp,
            tc.tile_pool(name="pp", bufs=3, space=bass.MemorySpace.PSUM) as pp,
            tc.tile_pool(name="pr", bufs=2, space=bass.MemorySpace.PSUM) as pr,
        ):
            xt, wih, whh, bias, xpt, cst = {}, {}, {}, {}, {}, {}
            for d in ("f", "b"):
                xt[d] = big.tile([128, 2 * TB], BF16, tag=f"xt{d}", name=f"xt{d}")
                nc.sync.dma_start(xt[d][:], xt_d[d][:])
                wih[d] = big.tile([128, 2048], BF16, tag=f"wih{d}", name=f"wih{d}")
                nc.sync.dma_start(wih[d][:], wih_d[d][:])
                whh[d] = big.tile([128, 2048], BF16, tag=f"whh{d}", name=f"whh{d}")
                nc.sync.dma_start(whh[d][:], whh_d[d][:])
                bias[d] = big.tile([128, 8], F32, tag=f"bias{d}", name=f"bias{d}")
                nc.sync.dma_start(bias[d][:], bias_d[d][:])
                xpt[d] = big.tile([128, t_steps * 8 * BC], BF16, tag=f"xpt{d}", name=f"xpt{d}")
                cst[d] = big.tile([128, 2 * BC], F32, tag=f"c{d}", name=f"c{d}")
                nc.vector.memset(cst[d][:], 0.0)
            zh = big.tile([128, 4 * BC], BF16, tag="zh", name="zh")
            nc.vector.memset(zh[:], 0.0)
            ident = big.tile([128, 128], BF16, tag="ident", name="ident")
            nc.sync.dma_start(ident[:], ident_d[:])

            # ---- phase 1: input projection (emitted lazily, paced into the
            # recurrence loop so it fills engine idle time instead of
            # blocking the first recurrence steps) ----
            # xpt layout per dir: col = j*TB + t*BC + b -> projection writes
            # are contiguous [128,512]; the recurrence I-MM reads a strided
            # [128, 8, BC] view.
            ncols = min(512, TB)
            ntiles = TB // ncols

            proj_groups = [
                (d, nt, j)
                for nt in range(ntiles)
                for d in ("f", "b")
                for j in range(8)
            ]

            def emit_proj_group(d, nt, j):
                ps = pp.tile([128, ncols], F32, tag="pp", name="pp")
                for kk in (0, 1):
                    nc.tensor.matmul(
                        ps[:],
                        wih[d][:, kk * 1024 + j * 128 : kk * 1024 + (j + 1) * 128],
                        xt[d][:, kk * TB + nt * ncols : kk * TB + (nt + 1) * ncols],
                        start=(kk == 0),
                        stop=(kk == 1),
                    )
                nc.vector.tensor_scalar(
                    xpt[d][:, j * TB + nt * ncols : j * TB + (nt + 1) * ncols],
                    ps[:], bias[d][:, j : j + 1], None, mybir.AluOpType.add,
                )

            # ---- phase 2: recurrence (staggered F/B emission) ----
            # psum(t) = I.T @ xp(t)  (start=True)  then += Whh tiles; the
            # sigmoid reads PSUM directly.  tanh(g) is folded into the wide
            # sigmoid: g rows were pre-scaled x2 on host, tanh(g)=2*sig(2g)-1.
            stg_tiles = {}

            def stg_slot(u):
                return stg_tiles[u // STG], (u % STG) * 4 * BC

            def emit_burst(d, doff, t):
                if t == 0:
                    prev = zh[:]
                else:
                    st, off = stg_slot(t - 1)
                    prev = st[:, off : off + 4 * BC]
                ps = pr.tile([128, 8 * BC], F32, tag=f"pr{d}", name=f"pr{d}")
                xv = xpt[d][:].rearrange("p (j tb) -> p j tb", j=8)
                nc.tensor.matmul(
                    ps[:], ident[:], xv[:, :, t * BC : (t + 1) * BC],
                    start=True, stop=False,
                )
                for kk in (0, 1):
                    rhs = prev[:, doff + kk * BC : doff + (kk + 1) * BC]
                    for j in range(8):
                        nc.tensor.matmul(
                            ps[:, j * BC : (j + 1) * BC],
                            whh[d][:, kk * 1024 + j * 128 : kk * 1024 + (j + 1) * 128],
                            rhs,
                            start=False,
                            stop=(j == 7 and kk == 1),
                        )
                return ps

            last_addc = {"f": None, "b": None}

            def emit_ew(d, doff, t, ps):
                st, off = stg_slot(t)
                other = "b" if d == "f" else "f"
                act = work.tile([128, 8 * BC], F32, tag=f"act{d}", name=f"act{d}")
                nc.scalar.activation(act[:], ps[:], AF.Sigmoid)
                g2 = work.tile([128, 2 * BC], F32, tag=f"g2{d}", name=f"g2{d}")
                i_ts = nc.vector.tensor_scalar(
                    g2[:], act[:, 6 * BC : 8 * BC], 2.0, -1.0,
                    mybir.AluOpType.mult, mybir.AluOpType.add,
                )
                if last_addc[other] is not None:
                    add_dep_helper(i_ts.ins, last_addc[other].ins, sync=False,
                                   reason="keep c-chains contiguous on DVE")
                tmp = work.tile([128, 2 * BC], F32, tag=f"tmp{d}", name=f"tmp{d}")
                nc.vector.tensor_mul(tmp[:], act[:, : 2 * BC], g2[:])
                fc = work.tile([128, 2 * BC], F32, tag=f"fc{d}", name=f"fc{d}")
                nc.vector.tensor_mul(fc[:], act[:, 2 * BC : 4 * BC], cst[d][:])
                cnew = work.tile([128, 2 * BC], F32, tag=f"c{d}", name=f"c{d}", bufs=2)
                last_addc[d] = nc.vector.tensor_add(cnew[:], fc[:], tmp[:])
                cst[d] = cnew
                th = work.tile([128, 2 * BC], F32, tag=f"th{d}", name=f"th{d}")
                nc.scalar.activation(th[:], cnew[:], AF.Tanh)
                nc.vector.tensor_mul(
                    st[:, off + doff : off + doff + BC],
                    act[:, 4 * BC : 5 * BC],
                    th[:, :BC],
                )
                nc.vector.tensor_mul(
                    st[:, off + doff + BC : off + doff + 2 * BC],
                    act[:, 5 * BC : 6 * BC],
                    th[:, BC:],
                )

            # upfront: the first two ntiles (steps 0..127); the rest paced
            gq = list(proj_groups)
            n_upfront = min(len(gq), 32)
            for _ in range(n_upfront):
                emit_proj_group(*gq.pop(0))
            n_rest = len(gq)

            ps_f = ps_b = None
            for t in range(t_steps):
                if n_rest:
                    tgt = min(n_rest, (t * n_rest) // max(1, (t_steps - 128)) + 1)
                    while len(gq) > n_rest - tgt:
                        emit_proj_group(*gq.pop(0))
                if t % STG == 0:
                    stg_tiles[t // STG] = stgp.tile(
                        [128, STG * 4 * BC], BF16, tag="stg", name="stg"
                    )
                ps_f = emit_burst("f", 0, t)
                if t >= 1:
                    emit_ew("b", 2 * BC, t - 1, ps_b)
                    if t % STG == 0:
                        blk = t // STG - 1
                        nc.sync.dma_start(
                            out_e[:, blk * STG * 4 * BC : (blk + 1) * STG * 4 * BC],
                            stg_tiles[blk][:],
                        )
                ps_b = emit_burst("b", 2 * BC, t)
                emit_ew("f", 0, t, ps_f)
            emit_ew("b", 2 * BC, t_steps - 1, ps_b)
            blk = nb - 1
            nc.sync.dma_start(
                out_e[:, blk * STG * 4 * BC : (blk + 1) * STG * 4 * BC],
                stg_tiles[blk][:],
            )

    nc.compile()
    return nc


def _prep_core(xs, Wih, Whh, bih, bhh, t_steps):
    """Host-side layout prep for one core, one direction.

    xs: [BC, t, D] f32 (already reversed for the backward direction).
    Returns dict of device arrays.
    """
    TB = t_steps * BC
    Wp = Wih[_PERM].astype(np.float32).copy()   # [1024, 256]
    Wh = Whh[_PERM].astype(np.float32).copy()
    bsum = (bih + bhh)[_PERM].astype(np.float32).copy()
    # tanh(g) is computed as 2*sigmoid(2g)-1 on device: pre-scale g rows x2
    Wp[3 * H :] *= 2.0
    Wh[3 * H :] *= 2.0
    bsum[3 * H :] *= 2.0

    def wt_layout(W):  # [4H, 256] -> [128, 2048] lhsT layout
        WT = W.T.reshape(2, 128, G4).transpose(1, 0, 2).reshape(128, 2 * G4)
        return np.ascontiguousarray(WT).astype(ml_dtypes.bfloat16)

    xT = (
        xs.transpose(2, 1, 0)                   # [256, t, BC]
        .reshape(2, 128, TB)
        .transpose(1, 0, 2)
        .reshape(128, 2 * TB)
    )
    return {
        "xt": np.ascontiguousarray(xT).astype(ml_dtypes.bfloat16),
        "wih": wt_layout(Wp),
        "whh": wt_layout(Wh),
        "bias": np.ascontiguousarray(bsum.reshape(8, 128).T).astype(np.float32),
    }


_NC_CACHE = {}


def _get_nc(t_steps):
    if t_steps not in _NC_CACHE:
        _NC_CACHE[t_steps] = build_nc(t_steps)
    return _NC_CACHE[t_steps]


def kernel(x, input_length, Wih_f, Whh_f, bih_f, bhh_f, Wih_b, Whh_b, bih_b, bhh_b,
           t_steps=T, _want_trace=False):
    x = np.asarray(x, np.float32)
    lens = np.asarray(input_length).astype(np.int64)
    L = t_steps
    tt = np.arange(L)

    nc = _get_nc(t_steps)

    in_maps = []
    for c in range(NCORES):
        bs = slice(c * BC, (c + 1) * BC)
        xs = x[bs, :L]
        ls = lens[bs]
        inv_idx = L - 1 - ((L - ls[:, None] + tt[None, :]) % L)       # [BC, L]
        xn = np.take_along_axis(xs, inv_idx[:, :, None], axis=1)
        pf = _prep_core(xs, Wih_f, Whh_f, bih_f, bhh_f, L)
        pb = _prep_core(xn, Wih_b, Whh_b, bih_b, bhh_b, L)
        in_maps.append(
            {
                "xt_f": pf["xt"], "wih_f": pf["wih"], "whh_f": pf["whh"], "bias_f": pf["bias"],
                "xt_b": pb["xt"], "wih_b": pb["wih"], "whh_b": pb["whh"], "bias_b": pb["bias"],
                "ident": np.eye(128, dtype=np.float32).astype(ml_dtypes.bfloat16),
            }
        )

    kw = {}
    if _want_trace:
        kw = dict(trace=True)
    res = run_bass_kernel_spmd(nc, in_maps, core_ids=list(range(NCORES)), **kw)

    outs = []
    for c in range(NCORES):
        bs = slice(c * BC, (c + 1) * BC)
        ls = lens[bs]
        arr = np.asarray(res.results[c]["out"]).astype(np.float32)
        arr = arr.reshape(128, L, 4, BC)
        fwd = arr[:, :, 0:2, :].transpose(3, 1, 2, 0).reshape(BC, L, 2 * 128)
        bwd = arr[:, :, 2:4, :].transpose(3, 1, 2, 0).reshape(BC, L, 2 * 128)
        bwd_idx = np.clip(ls[:, None] - 1 - tt[None, :], 0, L - 1)
        bwd_g = np.take_along_axis(bwd, bwd_idx[:, :, None], axis=1)
        o = np.concatenate([fwd, bwd_g], axis=-1)
        mask = (tt[None, :] < ls[:, None])[:, :, None]
        outs.append(np.where(mask, o, 0.0).astype(np.float32))
    full = np.concatenate(outs, axis=0)
    if _want_trace:
        return full, res
    return full



# revision 3
# speedup vs baseline: 1.0577x; 1.0577x over previous
"""BiLSTM Trainium2 kernel: B=64, T=512, D=256, H=256, 8 NeuronCores.

Sharding (v2, direction-split): cores 0-3 run the FORWARD direction on
batch quarters (16 seqs each); cores 4-7 run the BACKWARD direction on
the host-reversed input, same quarters.  One LSTM chain per core.

Why: the baseline ran both directions per core (8 seqs each), so every
recurrence matmul had only 8 moving columns against a 128-column
LDWEIGHTS.  One direction at 16 seqs halves the number of
weight-load+matmul pairs per core AND doubles the width of every
DVE/ACT elementwise op (fixed ~110-250ns overhead per op dominates at
these sizes).

Device (per core):
  phase 1 (paced into the loop): x projection xpT = WihT.T @ xT
      (N=512 matmuls; bias folded in via ScalarE Copy-with-bias)
  phase 2: 512-step recurrence, per step:
      2 ident matmuls inject xp(t) into PSUM (start=True), then 16
      weight-stationary matmuls (N=16) accumulate Whh @ h(t-1).
      PSUM is split ifg-bank/o-bank so the wide sigmoid over i,f,g can
      issue while the o-gate matmuls still run.
      EW: sig_ifg[128,96] (g pre-scaled x2 -> tanh via 2*sig-1),
      g2/tmp/fc/c' on DVE [128,32], tanh(c'), sig_o, h = sig_o*th.
  h states staged 32 steps at a time, DMA'd to DRAM as bf16.

Gate order is torch-native i,f,g,o (no permutation); only the g rows
are pre-scaled x2 on the host.
"""

import sys

for _p in ("/opt/trn_rl_repo",):
    if _p not in sys.path:
        sys.path.insert(0, _p)

import numpy as np
import ml_dtypes

import concourse.bass as bass
import concourse.mybir as mybir
import concourse.tile as tile
from concourse import bacc
from concourse.bass_utils import run_bass_kernel_spmd

B, T, D, H = 64, 512, 256, 256
NCORES = 8
BC = B // 4               # 16 sequences per core (4-way batch x 2-way direction)
G4 = 4 * H                # 1024 gate dims
STG = 32                  # recurrence steps per output staging block

BF16 = mybir.dt.bfloat16
F32 = mybir.dt.float32
AF = mybir.ActivationFunctionType


def build_nc(t_steps=T):
    assert t_steps % STG == 0
    nb = t_steps // STG
    TB = t_steps * BC  # (t,b) columns per k-half of xT

    nc = bacc.Bacc(None, target_bir_lowering=False)

    xt_d = nc.dram_tensor("xt", [128, 2 * TB], BF16, kind="ExternalInput")
    wih_d = nc.dram_tensor("wih", [128, 2048], BF16, kind="ExternalInput")
    whh_d = nc.dram_tensor("whh", [128, 2048], BF16, kind="ExternalInput")
    bias_d = nc.dram_tensor("bias", [128, 8], F32, kind="ExternalInput")
    ident_d = nc.dram_tensor("ident", [128, 128], BF16, kind="ExternalInput")
    out_e = nc.dram_tensor("out", [128, t_steps * 2 * BC], BF16, kind="ExternalOutput")

    with tile.TileContext(nc) as tc:
        with (
            tc.tile_pool(name="big", bufs=1) as big,
            tc.tile_pool(name="work", bufs=3) as work,
            tc.tile_pool(name="stgp", bufs=2) as stgp,
            tc.tile_pool(name="pp", bufs=2, space=bass.MemorySpace.PSUM) as pp,
            tc.tile_pool(name="pg", bufs=2, space=bass.MemorySpace.PSUM) as pg,
            tc.tile_pool(name="po", bufs=2, space=bass.MemorySpace.PSUM) as po,
        ):
            xt = big.tile([128, 2 * TB], BF16, tag="xt", name="xt")
            # chunked input DMA so projection can start early
            nxch = 4
            for c in range(nxch):
                w = 2 * TB // nxch
                nc.sync.dma_start(xt[:, c * w : (c + 1) * w], xt_d[:, c * w : (c + 1) * w])
            wih = big.tile([128, 2048], BF16, tag="wih", name="wih")
            nc.sync.dma_start(wih[:], wih_d[:])
            whh = big.tile([128, 2048], BF16, tag="whh", name="whh")
            nc.sync.dma_start(whh[:], whh_d[:])
            bias = big.tile([128, 8], F32, tag="bias", name="bias")
            nc.sync.dma_start(bias[:], bias_d[:])
            ident = big.tile([128, 128], BF16, tag="ident", name="ident")
            nc.sync.dma_start(ident[:], ident_d[:])
            xpt = big.tile([128, t_steps * 8 * BC], BF16, tag="xpt", name="xpt")
            cst = big.tile([128, 2 * BC], F32, tag="c0", name="c0")
            nc.vector.memset(cst[:], 0.0)
            zh = big.tile([128, 2 * BC], BF16, tag="zh", name="zh")
            nc.vector.memset(zh[:], 0.0)

            # ---- phase 1: input projection, paced into the recurrence ----
            # xpt col layout: j*TB + t*BC + b  (j = gate chunk 0..7)
            ncols = min(512, TB)
            ntiles = TB // ncols
            # nt-outer so early steps' xp is produced first
            proj_groups = [(nt, j) for nt in range(ntiles) for j in range(8)]
            proj_i = [0]

            def emit_proj_group(nt, j):
                ps = pp.tile([128, 512], F32, tag="pp", name="pp")
                for kk in (0, 1):
                    nc.tensor.matmul(
                        ps[:, :ncols],
                        wih[:, kk * 1024 + j * 128 : kk * 1024 + (j + 1) * 128],
                        xt[:, kk * TB + nt * ncols : kk * TB + (nt + 1) * ncols],
                        start=(kk == 0),
                        stop=(kk == 1),
                    )
                dst = xpt[:, j * TB + nt * ncols : j * TB + (nt + 1) * ncols]
                if (nt + j) % 2 == 0:
                    nc.scalar.activation(dst, ps[:, :ncols], AF.Identity, bias=bias[:, j : j + 1])
                else:
                    nc.vector.tensor_scalar(
                        dst, ps[:, :ncols], bias[:, j : j + 1], None, mybir.AluOpType.add
                    )

            # ---- phase 2: recurrence ----
            xv = None  # set after xpt exists
            stg_tiles = {}

            def stg_slot(u):
                return stg_tiles[u // STG], (u % STG) * 2 * BC

            def emit_step(t):
                nonlocal cst
                if t == 0:
                    prev = zh[:]
                else:
                    st, off = stg_slot(t - 1)
                    prev = st[:, off : off + 2 * BC]
                ps_g = pg.tile([128, 512], F32, tag="pg", name="pg")
                ps_o = po.tile([128, 512], F32, tag="po", name="po")
                xvv = xpt[:].rearrange("p (j tb) -> p j tb", j=8)
                # xp injection: i,f,g chunks -> ifg bank; o chunks -> o bank
                nc.tensor.matmul(
                    ps_g[:, : 6 * BC], ident[:], xvv[:, 0:6, t * BC : (t + 1) * BC],
                    start=True, stop=False,
                )
                nc.tensor.matmul(
                    ps_o[:, : 2 * BC], ident[:], xvv[:, 6:8, t * BC : (t + 1) * BC],
                    start=True, stop=False,
                )
                # Whh accumulation: j0..j5 (i,f,g) then j6,j7 (o)
                for kk in (0, 1):
                    rhs = prev[:, kk * BC : (kk + 1) * BC]
                    for j in range(6):
                        nc.tensor.matmul(
                            ps_g[:, j * BC : (j + 1) * BC],
                            whh[:, kk * 1024 + j * 128 : kk * 1024 + (j + 1) * 128],
                            rhs,
                            start=False,
                            stop=(kk == 1 and j == 5),
                        )
                for kk in (0, 1):
                    rhs = prev[:, kk * BC : (kk + 1) * BC]
                    for j in (6, 7):
                        nc.tensor.matmul(
                            ps_o[:, (j - 6) * BC : (j - 5) * BC],
                            whh[:, kk * 1024 + j * 128 : kk * 1024 + (j + 1) * 128],
                            rhs,
                            start=False,
                            stop=(kk == 1 and j == 7),
                        )
                # EW chain
                st, off = stg_slot(t)
                sig = work.tile([128, 6 * BC], F32, tag="sig", name="sig")
                nc.scalar.activation(sig[:], ps_g[:, : 6 * BC], AF.Sigmoid)
                g2 = work.tile([128, 2 * BC], F32, tag="g2", name="g2")
                nc.vector.tensor_scalar(
                    g2[:], sig[:, 4 * BC : 6 * BC], 2.0, -1.0,
                    mybir.AluOpType.mult, mybir.AluOpType.add,
                )
                tmp = work.tile([128, 2 * BC], F32, tag="tmp", name="tmp")
                nc.vector.tensor_mul(tmp[:], sig[:, : 2 * BC], g2[:])
                fc = work.tile([128, 2 * BC], F32, tag="fc", name="fc")
                nc.vector.tensor_mul(fc[:], sig[:, 2 * BC : 4 * BC], cst[:])
                cnew = work.tile([128, 2 * BC], F32, tag="c", name="c", bufs=2)
                nc.vector.tensor_add(cnew[:], fc[:], tmp[:])
                cst = cnew
                th = work.tile([128, 2 * BC], F32, tag="th", name="th")
                nc.scalar.activation(th[:], cnew[:], AF.Tanh)
                so = work.tile([128, 2 * BC], F32, tag="so", name="so")
                nc.scalar.activation(so[:], ps_o[:, : 2 * BC], AF.Sigmoid)
                nc.vector.tensor_mul(st[:, off : off + 2 * BC], so[:], th[:])

            # upfront projection groups cover the first 2 staging blocks
            gq = list(proj_groups)
            n_upfront = min(len(gq), 16)
            for _ in range(n_upfront):
                emit_proj_group(*gq.pop(0))
            n_rest = len(gq)

            for t in range(t_steps):
                if n_rest:
                    tgt = min(n_rest, (t * n_rest) // max(1, (t_steps - 96)) + 1)
                    while len(gq) > n_rest - tgt:
                        emit_proj_group(*gq.pop(0))
                if t % STG == 0:
                    stg_tiles[t // STG] = stgp.tile(
                        [128, STG * 2 * BC], BF16, tag="stg", name="stg"
                    )
                emit_step(t)
                if t % STG == STG - 1:
                    blk = t // STG
                    nc.sync.dma_start(
                        out_e[:, blk * STG * 2 * BC : (blk + 1) * STG * 2 * BC],
                        stg_tiles[blk][:],
                    )

    nc.compile()
    return nc


def _prep_core(xs, Wih, Whh, bih, bhh, t_steps):
    """Host-side layout prep for one core.

    xs: [BC, t, D] f32 (already reversed for the backward direction).
    """
    TB = t_steps * BC
    Wp = Wih.astype(np.float32).copy()   # [1024, 256], torch gate order i,f,g,o
    Wh = Whh.astype(np.float32).copy()
    bsum = (bih + bhh).astype(np.float32).copy()
    # tanh(g) computed as 2*sigmoid(2g)-1 on device: pre-scale g rows x2
    Wp[2 * H : 3 * H] *= 2.0
    Wh[2 * H : 3 * H] *= 2.0
    bsum[2 * H : 3 * H] *= 2.0

    def wt_layout(W):  # [1024, 256] -> [128, 2048] lhsT layout
        WT = W.T.reshape(2, 128, G4).transpose(1, 0, 2).reshape(128, 2 * G4)
        return np.ascontiguousarray(WT).astype(ml_dtypes.bfloat16)

    xT = (
        xs.transpose(2, 1, 0)                   # [256, t, BC]
        .reshape(2, 128, TB)
        .transpose(1, 0, 2)
        .reshape(128, 2 * TB)
    )
    return {
        "xt": np.ascontiguousarray(xT).astype(ml_dtypes.bfloat16),
        "wih": wt_layout(Wp),
        "whh": wt_layout(Wh),
        "bias": np.ascontiguousarray(bsum.reshape(8, 128).T).astype(np.float32),
        "ident": np.eye(128, dtype=np.float32).astype(ml_dtypes.bfloat16),
    }


_NC_CACHE = {}


def _get_nc(t_steps):
    if t_steps not in _NC_CACHE:
        _NC_CACHE[t_steps] = build_nc(t_steps)
    return _NC_CACHE[t_steps]


def kernel(x, input_length, Wih_f, Whh_f, bih_f, bhh_f, Wih_b, Whh_b, bih_b, bhh_b,
           t_steps=T, _want_trace=False):
    x = np.asarray(x, np.float32)
    lens = np.asarray(input_length).astype(np.int64)
    L = t_steps
    tt = np.arange(L)

    nc = _get_nc(t_steps)

    in_maps = []
    for c in range(NCORES):
        q = c % 4                      # batch quarter
        bs = slice(q * BC, (q + 1) * BC)
        xs = x[bs, :L]
        if c < 4:
            in_maps.append(_prep_core(xs, Wih_f, Whh_f, bih_f, bhh_f, L))
        else:
            ls = lens[bs]
            inv_idx = L - 1 - ((L - ls[:, None] + tt[None, :]) % L)   # [BC, L]
            xn = np.take_along_axis(xs, inv_idx[:, :, None], axis=1)
            in_maps.append(_prep_core(xn, Wih_b, Whh_b, bih_b, bhh_b, L))

    kw = {}
    if _want_trace:
        kw = dict(trace=True)
    res = run_bass_kernel_spmd(nc, in_maps, core_ids=list(range(NCORES)), **kw)

    outs = []
    for q in range(4):
        bs = slice(q * BC, (q + 1) * BC)
        ls = lens[bs]
        fa = np.asarray(res.results[q]["out"]).astype(np.float32)
        ba = np.asarray(res.results[q + 4]["out"]).astype(np.float32)
        fwd = fa.reshape(128, L, 2, BC).transpose(3, 1, 2, 0).reshape(BC, L, 2 * 128)
        bwd = ba.reshape(128, L, 2, BC).transpose(3, 1, 2, 0).reshape(BC, L, 2 * 128)
        bwd_idx = np.clip(ls[:, None] - 1 - tt[None, :], 0, L - 1)
        bwd_g = np.take_along_axis(bwd, bwd_idx[:, :, None], axis=1)
        o = np.concatenate([fwd, bwd_g], axis=-1)
        mask = (tt[None, :] < ls[:, None])[:, :, None]
        outs.append(np.where(mask, o, 0.0).astype(np.float32))
    full = np.concatenate(outs, axis=0)
    if _want_trace:
        return full, res
    return full
